# revision 1
# baseline (speedup 1.0000x reference)
"""Bahdanau attention kernel for Trainium2 (8 NeuronCores, data-parallel over batch).

Problem shapes: query [64,512], values [64,2048,512], W1/W2 [512,256],
b1/b2 [256], V [256,1], bV [1]; output context [64,512] fp32.

Strategy (per core, 8 local batches):
  - values cast to fp16 and pre-transposed to [D, S] per batch on the host;
    device DMAs are plain contiguous loads (the on-device alternatives — PE
    transpose or DMA xbar transpose — cost PE cycles, or deadlock against
    SBUF-to-SBUF broadcast DMAs, respectively).
  - projT[u, s] = W1^T @ valuesT via PE (fp16 in, fp32 PSUM accumulate),
    fused tanh(+proj_q bias+b1+b2) on ScalarE.
  - score[s] = V^T tanh(...) via PE matvec into a [1, S] PSUM row.
  - softmax: exp on ScalarE straight out of PSUM (scores are O(3), no max
    subtraction needed) with fused row-sum; normalization deferred to the end.
  - context^T[d] = sum_s attn[s]*valuesT[d, s] via DVE fused multiply-reduce
    (scalar_tensor_tensor) against a partition-broadcast attn row, split
    into s-halves so the broadcast overlaps the first half's reduce; final
    scale by 1/sum + PE transpose to row layout.

Measured/modeled per-core time: ~110 us single shot, ~94 us steady state
(TimelineSim cost model; HW wall-clock deltas bound it consistently).
Engine busy: PE 76.6 us (55 us irreducible fp16 values@W1 stream),
DVE 72.9, ACT 54.7, DMA 48.3, Pool 25.3.
"""

import sys

for _p in ("/opt/trn_rl_repo", "/opt/pypackages"):
    if _p not in sys.path:
        sys.path.insert(0, _p)

import numpy as np

import concourse.bacc as bacc
import concourse.mybir as mybir
from concourse.bass_utils import run_bass_kernel_spmd
from concourse.masks import make_identity
from concourse.tile import TileContext

N_CORES = 8
B, S, D, U = 64, 2048, 512, 256
BL = B // N_CORES  # local batches per core
DC = D // 128      # d-chunks
UC = U // 128      # u-chunks

F16 = mybir.dt.float16
F32 = mybir.dt.float32
AF = mybir.ActivationFunctionType
ALU = mybir.AluOpType


def build_bass(reps=1):
    # Bacc (not raw Bass): its compile() runs generate_event_semaphores(),
    # which splits multi-wait sync lists into the 1-wait-per-instruction
    # form TRN2 structs require.
    # reps>1 duplicates the whole compute body inside the NEFF — used only
    # for timing (per-rep time = (t(R)-t(1))/(R-1), cancelling the ~60 ms
    # fixed dispatch overhead of this execution path).
    nc = bacc.Bacc("TRN2", target_bir_lowering=False, debug=False)

    vt16 = nc.dram_tensor("vt16", [BL, D, S], F16, kind="ExternalInput").ap()
    w1r = nc.dram_tensor("w1r", [128, DC, U], F16, kind="ExternalInput").ap()
    qtr = nc.dram_tensor("qtr", [128, DC, BL], F16, kind="ExternalInput").ap()
    w2r = nc.dram_tensor("w2r", [128, DC, U], F16, kind="ExternalInput").ap()
    vr = nc.dram_tensor("vr", [128, UC, 1], F16, kind="ExternalInput").ap()
    b1r = nc.dram_tensor("b1r", [128, UC], F32, kind="ExternalInput").ap()
    ctx_out = nc.dram_tensor("ctx_out", [BL, D], F32, kind="ExternalOutput").ap()

    with TileContext(nc) as tc:
        with tc.tile_pool(name="const", bufs=1) as cpool, \
             tc.tile_pool(name="work", bufs=2) as wpool, \
             tc.tile_pool(name="psum", bufs=1, space="PSUM") as ppool:
            w1_sb = cpool.tile([128, DC, U], F16)
            nc.sync.dma_start(w1_sb, w1r)
            w2_sb = cpool.tile([128, DC, U], F16)
            nc.sync.dma_start(w2_sb, w2r)
            qt_sb = cpool.tile([128, DC, BL], F16)
            nc.sync.dma_start(qt_sb, qtr)
            v_sb = cpool.tile([128, UC, 1], F16)
            nc.sync.dma_start(v_sb, vr)
            b1_sb = cpool.tile([128, UC], F32)
            nc.sync.dma_start(b1_sb, b1r)
            # Re-copy b1 on ScalarE so the bias activation below has a
            # same-engine bias producer (keeps its sync-wait count low).
            b1_cp = cpool.tile([128, UC], F32)
            nc.scalar.activation(b1_cp, b1_sb, AF.Copy)
            ident = cpool.tile([128, 128], F32)
            make_identity(nc, ident)

            bias_sb = cpool.tile([128, UC, BL], F32)
            sume2_sb = cpool.tile([1, BL, 2], F32)
            sume_sb = cpool.tile([1, BL], F32)
            recip_sb = cpool.tile([1, BL], F32)
            recip_bc = cpool.tile([128, BL], F32)
            ctxT = cpool.tile([128, DC, BL], F32)
            ctxT2 = cpool.tile([128, DC, BL, 2], F32)
            ctxTs = cpool.tile([128, DC, BL], F32)
            ctx_fin = cpool.tile([DC * BL, 128], F32)

            for _rep in range(reps):
                # proj_q^T[u, b] + b1 + b2 (biases pre-combined into b1r)
                for uc in range(UC):
                    pq_ps = ppool.tile([128, BL], F32, tag="mainps", bufs=4,
                                       name=f"pq_ps_{_rep}_{uc}")
                    for dc in range(DC):
                        nc.tensor.matmul(
                            pq_ps,
                            w2_sb[:, dc, uc * 128:(uc + 1) * 128],
                            qt_sb[:, dc, :],
                            start=(dc == 0),
                            stop=(dc == DC - 1),
                        )
                    nc.scalar.activation(
                        bias_sb[:, uc, :], pq_ps, AF.Identity,
                        bias=b1_cp[:, uc:uc + 1]
                    )

                # Software pipeline: emit batch b's matmul/tanh stage, then
                # batch b-1's tail (score/exp/broadcast/context). The PE
                # thereby always has a full batch of dense matmul work
                # queued while ACT/Pool/DVE drain the previous batch.
                vTs, tanhs = {}, {}

                def emit_head(b, _rep=_rep):
                    # valuesT[d, s] for this batch (host pre-transposed);
                    # two DMAs (s-halves) so matmuls start on partial data.
                    vT = wpool.tile([128, DC, S], F16, tag="vT", bufs=4,
                                    name=f"vT_{_rep}_{b}")
                    vTs[b] = vT
                    src = vt16[b].rearrange("(dc p) s -> p dc s", p=128)
                    for quart in range(4):
                        sl = slice(quart * (S // 4), (quart + 1) * (S // 4))
                        nc.sync.dma_start(vT[:, :, sl], src[:, :, sl])

                    # projT[u, s] -> tanh(projT + bias) in fp16.
                    # dc inner-most-but-one so each W1 [128,128] stationary
                    # block serves 4 consecutive matmuls (ldweights reuse);
                    # the two 2-bank PSUM tiles hold all four 512-wide
                    # accumulation groups of this uc.
                    tanh_sb = wpool.tile([128, UC, S], F16, tag="tanh",
                                         bufs=4, name=f"tanh_sb_{_rep}_{b}")
                    tanhs[b] = tanh_sb
                    for uc in range(UC):
                        mm_ps = [
                            ppool.tile([128, 1024], F32, tag="mainps", bufs=4,
                                       name=f"mm_ps_{_rep}_{b}_{uc}_{sh}")
                            for sh in range(2)
                        ]
                        for dc in range(DC):
                            for sh in range(2):
                                for sq in range(2):
                                    nc.tensor.matmul(
                                        mm_ps[sh][:, sq * 512:(sq + 1) * 512],
                                        w1_sb[:, dc, uc * 128:(uc + 1) * 128],
                                        vT[:, dc,
                                           (sh * 2 + sq) * 512:
                                           (sh * 2 + sq + 1) * 512],
                                        start=(dc == 0),
                                        stop=(dc == DC - 1),
                                    )
                        for sh in range(2):
                            nc.scalar.activation(
                                tanh_sb[:, uc, sh * 1024:(sh + 1) * 1024],
                                mm_ps[sh],
                                AF.Tanh,
                                bias=bias_sb[:, uc, b:b + 1],
                            )

                def emit_tail(b, _rep=_rep):
                    vT, tanh_sb = vTs.pop(b), tanhs.pop(b)
                    # score row [1, S] = V^T @ tanh; four 512-wide quarters
                    # (1 PSUM bank each, double-buffered) + exp with fused
                    # partial row-sums.
                    attn_row = wpool.tile([1, S], F16,
                                          name=f"attn_row_{_rep}_{b}",
                                          tag=f"attnrow{b}", bufs=1)
                    for h in range(2):
                        sc_ps = ppool.tile([1, 1024], F32, tag="mainps",
                                           bufs=4,
                                           name=f"sc_ps_{_rep}_{b}_{h}")
                        for sc in range(2):
                            col = h * 1024 + sc * 512
                            for uc in range(UC):
                                nc.tensor.matmul(
                                    sc_ps[:, sc * 512:(sc + 1) * 512],
                                    v_sb[:, uc, :],
                                    tanh_sb[:, uc, col:col + 512],
                                    start=(uc == 0),
                                    stop=(uc == UC - 1),
                                )
                        nc.scalar.activation(
                            attn_row[:, h * 1024:(h + 1) * 1024],
                            sc_ps, AF.Exp,
                            accum_out=sume2_sb[:, b, h:h + 1],
                        )

                    # context^T[d] += sum_s attn[s] * vT[d, s] on DVE.
                    # scalar_tensor_tensor: out = (in0*1)*in1, accum = sum.
                    # (InstTensorTensorReduce hangs this runtime; this
                    # struct does the same fused multiply-reduce and works.)
                    attn_bc = wpool.tile([128, S], F16, tag="attnbc", bufs=3,
                                         name=f"attn_bc_{_rep}_{b}")
                    H = S // 2
                    for h in range(2):
                        nc.gpsimd.partition_broadcast(
                            attn_bc[:, h * H:(h + 1) * H],
                            attn_row[:, h * H:(h + 1) * H])
                    ttr_out = wpool.tile([128, H], F16, tag="ttrout",
                                         name=f"ttr_out_{_rep}_{b}")
                    for h in range(2):
                        for dc in range(DC):
                            nc.vector.scalar_tensor_tensor(
                                out=ttr_out,
                                in0=vT[:, dc, h * H:(h + 1) * H],
                                scalar=1.0,
                                in1=attn_bc[:, h * H:(h + 1) * H],
                                op0=ALU.mult,
                                op1=ALU.mult,
                                accum_out=ctxT2[:, dc, b, h:h + 1],
                            )

                for b in range(BL):
                    emit_head(b)
                    emit_tail(b)

                # combine context half-sums, then normalize and emit
                nc.vector.tensor_reduce(
                    ctxT, ctxT2, axis=mybir.AxisListType.X, op=ALU.add
                )
                nc.vector.tensor_reduce(
                    sume_sb, sume2_sb, axis=mybir.AxisListType.X, op=ALU.add
                )
                nc.vector.reciprocal(recip_sb, sume_sb)
                nc.gpsimd.partition_broadcast(recip_bc, recip_sb)
                for dc in range(DC):
                    nc.vector.tensor_tensor(
                        ctxTs[:, dc, :], ctxT[:, dc, :], recip_bc, ALU.mult
                    )
                ctx_ps = ppool.tile([DC * BL, 128], F32, tag="mainps", bufs=4,
                                    name=f"ctx_ps_{_rep}")
                nc.tensor.transpose(
                    ctx_ps, ctxTs.rearrange("p a b -> p (a b)"), ident
                )
                nc.vector.tensor_copy(ctx_fin, ctx_ps)
                for dc in range(DC):
                    nc.sync.dma_start(
                        ctx_out[:, dc * 128:(dc + 1) * 128],
                        ctx_fin[dc * BL:(dc + 1) * BL, :],
                    )

    nc.compile()
    return nc


_NC_CACHE = {}


def _get_nc(reps=1):
    if reps not in _NC_CACHE:
        _NC_CACHE[reps] = build_bass(reps)
    return _NC_CACHE[reps]


def make_in_maps(query, values, W1, b1, W2, b2, V, bV):
    """Host-side sharding + layout prep. bV drops out (softmax shift-invariance)."""
    del bV
    vt16 = np.ascontiguousarray(values.astype(np.float16).transpose(0, 2, 1))
    w1r = np.ascontiguousarray(
        W1.astype(np.float16).reshape(DC, 128, U).transpose(1, 0, 2)
    )
    w2r = np.ascontiguousarray(
        W2.astype(np.float16).reshape(DC, 128, U).transpose(1, 0, 2)
    )
    vr = np.ascontiguousarray(
        V.astype(np.float16).reshape(UC, 128, 1).transpose(1, 0, 2)
    )
    b1r = np.ascontiguousarray((b1 + b2).astype(np.float32).reshape(UC, 128).T)
    in_maps = []
    for c in range(N_CORES):
        q_loc = query[c * BL:(c + 1) * BL]  # [BL, D]
        qtr = np.ascontiguousarray(
            q_loc.T.astype(np.float16).reshape(DC, 128, BL).transpose(1, 0, 2)
        )
        in_maps.append({
            "vt16": vt16[c * BL:(c + 1) * BL],
            "w1r": w1r,
            "qtr": qtr,
            "w2r": w2r,
            "vr": vr,
            "b1r": b1r,
        })
    return in_maps


def run(trace=False, **inputs):
    nc = _get_nc()
    in_maps = make_in_maps(**{k: np.asarray(v) for k, v in inputs.items()})
    res = run_bass_kernel_spmd(
        nc, in_maps, core_ids=list(range(N_CORES)), trace=trace
    )
    out = np.concatenate(
        [res.results[c]["ctx_out"] for c in range(N_CORES)], axis=0
    )
    return out.astype(np.float32), res


def kernel(**inputs) -> np.ndarray:
    out, _ = run(trace=False, **inputs)
    return out


def _bench_fn(nc):
    """Build a jitted 8-core executor for a prebuilt nc with staged inputs."""
    import jax
    from jax.sharding import Mesh, NamedSharding, PartitionSpec
    from jax.experimental.shard_map import shard_map
    from concourse import bass2jax
    from concourse.bass2jax import _bass_exec_p, install_neuronx_cc_hook

    install_neuronx_cc_hook()
    partition_name = (
        nc.partition_id_tensor.name if nc.partition_id_tensor else None
    )
    in_names, out_names, out_avals, zero_outs = [], [], [], []
    for alloc in nc.m.functions[0].allocations:
        if not isinstance(alloc, mybir.MemoryLocationSet):
            continue
        name = alloc.memorylocations[0].name
        if alloc.kind == "ExternalInput":
            if name != partition_name:
                in_names.append(name)
        elif alloc.kind == "ExternalOutput":
            out_names.append(name)
            shape = tuple(alloc.tensor_shape)
            dtype = mybir.dt.np(alloc.dtype)
            out_avals.append(jax.core.ShapedArray(shape, dtype))
            zero_outs.append(np.zeros(shape, dtype))
    n_params = len(in_names)
    all_names = list(in_names) + list(out_names)
    if partition_name is not None:
        all_names.append(partition_name)

    def _body(*args):
        operands = list(args)
        if partition_name is not None:
            operands.append(bass2jax.partition_id_tensor())
        outs = _bass_exec_p.bind(
            *operands,
            out_avals=tuple(out_avals),
            in_names=tuple(all_names),
            out_names=tuple(out_names),
            lowering_input_output_aliases=(),
            sim_require_finite=True,
            sim_require_nnan=True,
            nc=nc,
        )
        return tuple(outs)

    devices = jax.devices()[:N_CORES]
    mesh = Mesh(np.asarray(devices), ("core",))
    n_io = n_params + len(out_names)
    fn = jax.jit(
        shard_map(_body, mesh=mesh,
                  in_specs=(PartitionSpec("core"),) * n_io,
                  out_specs=(PartitionSpec("core"),) * len(out_names),
                  check_rep=False),
        keep_unused=True,
    )
    return fn, in_names, zero_outs, mesh


def bench(big_reps=17, n_samples=10, **inputs):
    """Per-rep device time from in-NEFF repetition: (t(R)-t(1))/(R-1)."""
    import time

    import jax
    from jax.sharding import NamedSharding, PartitionSpec

    in_maps = make_in_maps(**{k: np.asarray(v) for k, v in inputs.items()})

    results = {}
    for reps in (1, big_reps):
        nc = _get_nc(reps)
        fn, in_names, zero_outs, mesh = _bench_fn(nc)
        concat_in = [
            np.concatenate([in_maps[c][n] for c in range(N_CORES)], axis=0)
            for n in in_names
        ]
        concat_zeros = [
            np.zeros((N_CORES * z.shape[0], *z.shape[1:]), z.dtype)
            for z in zero_outs
        ]
        sh = NamedSharding(mesh, PartitionSpec("core"))
        staged = [jax.device_put(a, sh) for a in concat_in + concat_zeros]
        out = fn(*staged)  # warmup: compile + first exec
        jax.block_until_ready(out)
        samples = []
        for _ in range(n_samples):
            t0 = time.perf_counter()
            out = fn(*staged)
            jax.block_until_ready(out)
            samples.append(time.perf_counter() - t0)
        samples.sort()
        results[reps] = samples

    # median-of-lower-half for robustness
    def est(v):
        lo = v[: max(1, len(v) // 2)]
        return sum(lo) / len(lo)

    per_rep = (est(results[big_reps]) - est(results[1])) / (big_reps - 1)
    return per_rep * 1e9, results

